# revision 17
# baseline (speedup 1.0000x reference)
"""Seq2seq RNN with attention on 8 TRN2 NeuronCores.

Strategy v2:
- Host gathers embeddings (x = E[idx] + bias) and pre-transposes to the
  on-device layout, so the device never touches the 32000-row tables.
- Every core redundantly runs the full-batch (B=32) encoder and decoder
  *recurrences only* -- the per-step critical path is just
  PE(matmuls into PSUM, with x/bias injected via identity/ones matmuls)
  -> ACT(tanh).  No DVE on the critical path.
- Attention is computed *after* the decoder scan, batched over all
  timesteps (it does not feed back into the recurrence).
- The final vocab projection is tensor-parallel over V: core i computes
  columns [i*4000, (i+1)*4000) for the full batch, so each core loads
  only 2 MB of W and writes its 1/8 slice of the logits (bf16).
Output rows are (b, t)-major; the host reassembles (T, B, V) f32.
"""

import numpy as np

import concourse.bass as bass
import concourse.bacc as bacc
import concourse.tile as tile
from concourse import mybir
from concourse.bass_utils import run_bass_kernel_spmd
from concourse.masks import make_identity

D = 256
V = 32000
T = 128  # T_SRC == T_TGT == 128
B = 32
NCORES = 8
VL = V // NCORES  # 4000 vocab cols per core
KC = D // 128  # 2 d-chunks of 128
NVB = 8  # vocab col-blocks per core
VB = VL // NVB  # 500 cols per block (fits one PSUM bank in f32)
DT = mybir.dt.float32
BF = mybir.dt.bfloat16
NPBF = mybir.dt.np(BF)
AF = mybir.ActivationFunctionType
ALU = mybir.AluOpType

_CACHE = {}


def _build(with_b2):
    nc = bacc.Bacc(None)

    u_d = nc.declare_dram_parameter("u", [D, D], BF, isOutput=False)
    cwt_d = nc.declare_dram_parameter("cwt", [D, D], BF, isOutput=False)
    w_d = nc.declare_dram_parameter("w_slice", [D, VL], BF, isOutput=False)
    xs_d = nc.declare_dram_parameter("xs", [128, T, KC, B], BF, isOutput=False)
    xt_d = nc.declare_dram_parameter("xt", [128, T, KC, B], BF, isOutput=False)
    b2r_d = nc.declare_dram_parameter("b2row", [1, D], BF, isOutput=False)
    b2c_d = nc.declare_dram_parameter("b2col", [128, KC], DT, isOutput=False)
    madd_d = nc.declare_dram_parameter("madd", [1, B * T], BF, isOutput=False)
    out_d = nc.declare_dram_parameter("out", [B * T, VL], BF, isOutput=True)

    with tile.TileContext(nc) as tc:
        with (
            tc.tile_pool(name="persist", bufs=1) as pp,
            tc.tile_pool(name="stage", bufs=4) as sp,
        ):
            # ---- persistent SBUF tiles ----
            u_sb = pp.tile([128, KC, D], BF, tag="u")
            cwt_sb = pp.tile([128, KC, D], BF, tag="cwt")
            w_sb = pp.tile([128, KC, VL], BF, tag="w")
            ident = pp.tile([128, 128], DT, tag="ident")
            identb = pp.tile([128, 128], BF, tag="identb")
            ones_b = pp.tile([1, 128], BF, tag="ones")
            b2r_sb = pp.tile([1, D], BF, tag="b2r")
            b2c_sb = pp.tile([128, KC], DT, tag="b2c")
            madd_sb = pp.tile([1, B * T], BF, tag="madd")
            # x'/h' layouts: [d_lo, k, b, t]
            xs = pp.tile([128, T, KC, B], BF, tag="xs")
            xt = pp.tile([128, T, KC, B], BF, tag="xt")
            hd2 = pp.tile([128, 2, KC, B], BF, tag="hd2")  # enc l2 state (dbl-buf)
            hdc = pp.tile([128, 2, KC, B], BF, tag="hdc")  # dec state (dbl-buf)
            hd1 = pp.tile([128, KC, B], BF, tag="hd1")  # enc layer-1 state
            he = pp.tile([128, KC, B, T], BF, tag="he")  # enc H'
            hd = pp.tile([128, KC, B, T], BF, tag="hd")  # dec h'
            het = pp.tile([128, B, KC, 128], BF, tag="het")  # He_T[ts, b, k, d]
            ctx = pp.tile([128, KC, B, T], BF, tag="ctx")  # ctx'
            houts = pp.tile([128, KC, B, 2, 64], BF, tag="houts")  # [d,k,b,H,t']

            # ---- load constants ----
            # Spread input DMAs over three issuing queues so the transfers
            # run in parallel: Sync carries what the encoder needs first,
            # DVE carries x_tgt, ACT carries W/cwt/madd (needed later).
            nc.sync.dma_start(out=xs[:, 0:16, :, :], in_=xs_d[:, 0:16, :, :])
            for k in range(KC):
                nc.sync.dma_start(out=u_sb[:, k, :], in_=u_d[k * 128:(k + 1) * 128, :])
            nc.sync.dma_start(out=b2c_sb[:, :], in_=b2c_d[:, :])
            nc.sync.dma_start(out=b2r_sb[:, :], in_=b2r_d[:, :])
            nc.sync.dma_start(out=xs[:, 16:, :, :], in_=xs_d[:, 16:, :, :])
            nc.gpsimd.dma_start(out=xt[:, :, :, :], in_=xt_d[:, :, :, :])
            for k in range(KC):
                nc.sync.dma_start(out=w_sb[:, k, :],
                                  in_=w_d[k * 128:(k + 1) * 128, :])
            for k in range(KC):
                nc.sync.dma_start(out=cwt_sb[:, k, :],
                                  in_=cwt_d[k * 128:(k + 1) * 128, :])
            nc.sync.dma_start(out=madd_sb[:, :], in_=madd_d[:, :])
            make_identity(nc, ident[:, :])
            nc.vector.tensor_copy(out=identb[:, :], in_=ident[:, :])
            nc.gpsimd.memset(ones_b[:, :], 1.0)

            # =============== encoder scan ===============
            with (
                tc.tile_pool(name="pe1", bufs=2, space="PSUM") as pe1,
                tc.tile_pool(name="pe2", bufs=2, space="PSUM") as pe2,
            ):
                for t in range(T):
                    if t == 0:
                        # h1_0 = tanh(x_0)  (x includes b1)
                        nc.scalar.activation(out=hd1[:, :, :], in_=xs[:, 0, :, :],
                                             func=AF.Tanh)
                        # h2_0 = tanh(h1_0 + b2)
                        for m in range(KC):
                            nc.scalar.activation(out=hd2[:, 0, m, :],
                                                 in_=hd1[:, m, :], func=AF.Tanh,
                                                 bias=b2c_sb[:, m:m + 1])
                        nc.gpsimd.tensor_copy(out=he[:, :, :, 0],
                                              in_=hd2[:, 0, :, :])
                        continue
                    # ---- layer 1: p1 = x_t (+b1, folded) + U.T-chunks @ h1 ----
                    p1 = pe1.tile([128, KC, B], DT, tag="p1")
                    nc.tensor.matmul(out=p1[:, :, :], lhsT=identb[:, :],
                                     rhs=xs[:, t, :, :], start=True, stop=False,
                                     skip_group_check=True)
                    for m in range(KC):
                        for k in range(KC):
                            nc.tensor.matmul(
                                out=p1[:, m, :],
                                lhsT=u_sb[:, k, m * 128:(m + 1) * 128],
                                rhs=hd1[:, k, :],
                                start=False, stop=(k == KC - 1),
                                skip_group_check=True)
                    nc.scalar.activation(out=hd1[:, :, :], in_=p1[:, :, :],
                                         func=AF.Tanh)
                    # ---- layer 2: p2 = b2 + h1_t + U.T-chunks @ h2 ----
                    p2 = pe2.tile([128, KC, B], DT, tag="p2")
                    if with_b2:
                        for m in range(KC):
                            nc.tensor.matmul(out=p2[:, m, :],
                                             lhsT=b2r_sb[:, m * 128:(m + 1) * 128],
                                             rhs=ones_b[:, 0:B],
                                             start=(m == 0), stop=False,
                                             skip_group_check=True)
                    for m in range(KC):
                        for k in range(KC):
                            nc.tensor.matmul(
                                out=p2[:, m, :],
                                lhsT=u_sb[:, k, m * 128:(m + 1) * 128],
                                rhs=hd2[:, (t - 1) % 2, k, :],
                                start=(not with_b2 and m == 0 and k == 0),
                                stop=False,
                                skip_group_check=True)
                    nc.tensor.matmul(out=p2[:, :, :], lhsT=identb[:, :],
                                     rhs=hd1[:, :, :], start=False, stop=True,
                                     skip_group_check=True)
                    nc.scalar.activation(out=hd2[:, t % 2, :, :],
                                         in_=p2[:, :, :], func=AF.Tanh)
                    nc.gpsimd.tensor_copy(out=he[:, :, :, t],
                                          in_=hd2[:, t % 2, :, :])

            # =============== decoder scan (+ interleaved He transposes) ======
            with (
                tc.tile_pool(name="pd", bufs=2, space="PSUM") as pd,
                tc.tile_pool(name="pt", bufs=2, space="PSUM") as pt,
            ):
                tp_jobs = [(b, m) for b in range(B) for m in range(KC)]  # 64
                for t in range(T):
                    p = pd.tile([128, KC, B], DT, tag="pdec")
                    nc.tensor.matmul(out=p[:, :, :], lhsT=identb[:, :],
                                     rhs=xt[:, t, :, :], start=True, stop=False,
                                     skip_group_check=True)
                    for m in range(KC):
                        for k in range(KC):
                            prev = (hd2[:, (T - 1) % 2, k, :] if t == 0
                                    else hdc[:, (t - 1) % 2, k, :])
                            nc.tensor.matmul(
                                out=p[:, m, :],
                                lhsT=u_sb[:, k, m * 128:(m + 1) * 128],
                                rhs=prev,
                                start=False, stop=(k == KC - 1),
                                skip_group_check=True)
                    nc.scalar.activation(out=hdc[:, t % 2, :, :],
                                         in_=p[:, :, :], func=AF.Tanh)
                    nc.gpsimd.tensor_copy(out=hd[:, :, :, t],
                                          in_=hdc[:, t % 2, :, :])
                    # one He_T transpose every other step fills PE idle time
                    if t % 2 == 1 and tp_jobs:
                        b, m = tp_jobs.pop(0)
                        tps = pt.tile([128, 128], BF, tag="tps")
                        nc.tensor.transpose(tps[:, :], he[:, m, b, :],
                                            identb[:, :])
                        nc.vector.tensor_copy(out=het[:, b, m, :],
                                              in_=tps[:, :])

            # =============== attention (batched over t) ===============
            with (
                tc.tile_pool(name="ps", bufs=3, space="PSUM") as ps,
                tc.tile_pool(name="pa", bufs=2, space="PSUM") as pa,
                tc.tile_pool(name="pc", bufs=3, space="PSUM") as pc,
                tc.tile_pool(name="watt", bufs=4) as watt,
            ):
                for b in range(B):
                    # scores S[tt, ts] = mask + sum_d hd[d, tt] he[d, ts]
                    s_ps = ps.tile([128, 128], DT, tag="sps")
                    nc.tensor.matmul(out=s_ps[:, :], lhsT=ones_b[:, :],
                                     rhs=madd_sb[:, b * T:(b + 1) * T],
                                     start=True, stop=False,
                                     skip_group_check=True)
                    for k in range(KC):
                        nc.tensor.matmul(out=s_ps[:, :], lhsT=hd[:, k, b, :],
                                         rhs=he[:, k, b, :],
                                         start=False, stop=(k == KC - 1),
                                         skip_group_check=True)
                    # softmax over ts (free axis), scale 1/16 inside exp
                    ex = watt.tile([128, 128], BF, tag="ex")
                    sm = watt.tile([128, 1], DT, tag="sm")
                    nc.scalar.activation(out=ex[:, :], in_=s_ps[:, :],
                                         func=AF.Exp, scale=1.0 / 16.0,
                                         accum_out=sm[:, :])
                    rs = watt.tile([128, 1], DT, tag="rs")
                    nc.vector.reciprocal(out=rs[:, :], in_=sm[:, :])
                    alpha = watt.tile([128, 128], BF, tag="alpha")
                    nc.vector.tensor_scalar(out=alpha[:, :], in0=ex[:, :],
                                            scalar1=rs[:, :1], scalar2=None,
                                            op0=ALU.mult)
                    # alpha [tt, ts] -> alphaT [ts, tt]
                    a_ps = pa.tile([128, 128], BF, tag="aps")
                    nc.tensor.transpose(a_ps[:, :], alpha[:, :], identb[:, :])
                    a_t = watt.tile([128, 128], BF, tag="at")
                    nc.scalar.copy(out=a_t[:, :], in_=a_ps[:, :])
                    # ctx'[d_m, tt] = He_T[ts, d_m].T @ alphaT[ts, tt]
                    for m in range(KC):
                        c_ps = pc.tile([128, 128], DT, tag="cps")
                        nc.tensor.matmul(out=c_ps[:, :], lhsT=het[:, b, m, :],
                                         rhs=a_t[:, :], start=True, stop=True)
                        nc.vector.tensor_copy(out=ctx[:, m, b, :],
                                              in_=c_ps[:, :])

            # ---- outs = hd + ctx@ctx_W.T, then vocab projection ----
            # g-outer so each group of 4 batches' outs lands right before
            # its projection; the DVE adds spread across the proj phase.
            with (
                tc.tile_pool(name="po", bufs=2, space="PSUM") as po,
                tc.tile_pool(name="pl", bufs=4, space="PSUM") as pl,
            ):
                for g in range(8):
                    for m in range(KC):
                        o_ps = po.tile([128, 512], DT, tag="ops")
                        for k in range(KC):
                            nc.tensor.matmul(
                                out=o_ps[:, :],
                                lhsT=cwt_sb[:, k, m * 128:(m + 1) * 128],
                                rhs=ctx[:, k, g * 4:(g + 1) * 4, :],
                                start=(k == 0), stop=(k == KC - 1))
                        nc.vector.tensor_add(
                            out=houts[:, m, g * 4:(g + 1) * 4, :, :],
                            in0=o_ps[:, :],
                            in1=hd[:, m, g * 4:(g + 1) * 4, :])
                    for b in range(g * 4, (g + 1) * 4):
                        stg = sp.tile([128, VL], BF, tag="stg")
                        for vb in range(NVB):
                            l_ps = pl.tile([128, VB], DT, tag="lps")
                            for k in range(KC):
                                nc.tensor.matmul(
                                    out=l_ps[:, :],
                                    lhsT=houts[:, k, b, :, :],
                                    rhs=w_sb[:, k, vb * VB:(vb + 1) * VB],
                                    start=(k == 0), stop=(k == KC - 1))
                            if vb % 2 == 0:
                                nc.vector.tensor_copy(
                                    out=stg[:, vb * VB:(vb + 1) * VB],
                                    in_=l_ps[:, :])
                            else:
                                nc.scalar.copy(
                                    out=stg[:, vb * VB:(vb + 1) * VB],
                                    in_=l_ps[:, :])
                            if vb == 3:
                                nc.sync.dma_start(
                                    out=out_d[b * T:(b + 1) * T, 0:4 * VB],
                                    in_=stg[:, 0:4 * VB])
                            elif b >= 28 and vb in (5, 7):
                                lo2 = (4 if vb == 5 else 6) * VB
                                nc.sync.dma_start(
                                    out=out_d[b * T:(b + 1) * T,
                                              lo2:lo2 + 2 * VB],
                                    in_=stg[:, lo2:lo2 + 2 * VB])
                        if b < 28:
                            nc.sync.dma_start(
                                out=out_d[b * T:(b + 1) * T, 4 * VB:],
                                in_=stg[:, 4 * VB:])
    nc.compile()
    return nc


def _prep_in_maps(U, b_enc1, b_enc2, b_dec, E_en, E_de, ctx_W, W_out_de,
                  src_en, tgt_de_in):
    f32 = np.float32
    U = np.ascontiguousarray(U, f32).astype(NPBF)
    cwt = np.ascontiguousarray(np.asarray(ctx_W, f32).T).astype(NPBF)
    w_t = np.ascontiguousarray(np.asarray(W_out_de, f32).T)  # [D, V] f32
    E_en = np.asarray(E_en, f32)
    E_de = np.asarray(E_de, f32)
    src = np.asarray(src_en)
    tgt = np.asarray(tgt_de_in)

    def gather_x(E, idx, bias):
        # x'[p, k, b, t] = E[idx[t, b], k*128 + p] + bias[k*128+p]
        x = E[idx] + np.asarray(bias, f32)  # (T, B, D)
        x = x.transpose(2, 0, 1).reshape(KC, 128, T, B)  # (k, p, t, b)
        x = x.transpose(1, 2, 0, 3)  # (p, t, k, b)
        return np.ascontiguousarray(x).astype(NPBF)

    xs = gather_x(E_en, src, b_enc1)
    xt = gather_x(E_de, tgt, b_dec)
    b2row = np.asarray(b_enc2, f32).reshape(1, D).astype(NPBF)
    b2col = np.ascontiguousarray(np.asarray(b_enc2, f32).reshape(KC, 128).T)
    madd = np.where(src == 0, f32(-1e9), f32(0.0)).T.reshape(1, B * T)  # (b,t)
    madd = madd.astype(NPBF)

    in_maps = []
    for i in range(NCORES):
        in_maps.append({
            "u": U, "cwt": cwt,
            "w_slice": np.ascontiguousarray(
                w_t[:, i * VL:(i + 1) * VL]).astype(NPBF),
            "xs": xs, "xt": xt,
            "b2row": b2row, "b2col": b2col, "madd": madd,
        })
    return in_maps


def kernel(U, b_enc1, b_enc2, b_dec, E_en, E_de, ctx_W, W_out_de,
           src_en, tgt_de_in, _trace=False, _raw=False):
    with_b2 = bool(np.any(np.asarray(b_enc2) != 0))
    key = ("nc", with_b2)
    if key not in _CACHE:
        _CACHE[key] = _build(with_b2)
    nc = _CACHE[key]
    in_maps = _prep_in_maps(U, b_enc1, b_enc2, b_dec, E_en, E_de, ctx_W,
                            W_out_de, src_en, tgt_de_in)
    res = run_bass_kernel_spmd(nc, in_maps, list(range(NCORES)), trace=_trace)
    if _raw:
        return res
    logits = np.empty((T, B, V), np.float32)
    for i in range(NCORES):
        blk = res.results[i]["out"].astype(np.float32).reshape(B, T, VL)
        logits[:, :, i * VL:(i + 1) * VL] = blk.transpose(1, 0, 2)
    if _trace:
        return logits, res
    return logits


# revision 18
# speedup vs baseline: 1.1911x; 1.1911x over previous
"""Seq2seq RNN with attention on 8 TRN2 NeuronCores.

Strategy v2:
- Host gathers embeddings (x = E[idx] + bias) and pre-transposes to the
  on-device layout, so the device never touches the 32000-row tables.
- Every core redundantly runs the full-batch (B=32) encoder and decoder
  *recurrences only* -- the per-step critical path is just
  PE(matmuls into PSUM, with x/bias injected via identity/ones matmuls)
  -> ACT(tanh).  No DVE on the critical path.
- Attention is computed *after* the decoder scan, batched over all
  timesteps (it does not feed back into the recurrence).
- The final vocab projection is tensor-parallel over V: core i computes
  columns [i*4000, (i+1)*4000) for the full batch, so each core loads
  only 2 MB of W and writes its 1/8 slice of the logits (bf16).
Output rows are (b, t)-major; the host reassembles (T, B, V) f32.
"""

import numpy as np

import concourse.bass as bass
import concourse.bacc as bacc
import concourse.tile as tile
from concourse import mybir
from concourse.bass_utils import run_bass_kernel_spmd
from concourse.masks import make_identity

D = 256
V = 32000
T = 128  # T_SRC == T_TGT == 128
B = 32
NCORES = 8
VL = V // NCORES  # 4000 vocab cols per core
KC = D // 128  # 2 d-chunks of 128
NVB = 8  # vocab col-blocks per core
VB = VL // NVB  # 500 cols per block (fits one PSUM bank in f32)
DT = mybir.dt.float32
BF = mybir.dt.bfloat16
NPBF = mybir.dt.np(BF)
AF = mybir.ActivationFunctionType
ALU = mybir.AluOpType

_CACHE = {}


def _build(with_b2):
    nc = bacc.Bacc(None)

    u_d = nc.declare_dram_parameter("u", [D, D], BF, isOutput=False)
    cwt_d = nc.declare_dram_parameter("cwt", [D, D], BF, isOutput=False)
    w_d = nc.declare_dram_parameter("w_slice", [D, VL], BF, isOutput=False)
    xs_d = nc.declare_dram_parameter("xs", [128, T, KC, B], BF, isOutput=False)
    xt_d = nc.declare_dram_parameter("xt", [128, T, KC, B], BF, isOutput=False)
    b2r_d = nc.declare_dram_parameter("b2row", [1, D], BF, isOutput=False)
    b2c_d = nc.declare_dram_parameter("b2col", [128, KC], DT, isOutput=False)
    madd_d = nc.declare_dram_parameter("madd", [1, B * T], BF, isOutput=False)
    out_d = nc.declare_dram_parameter("out", [B * T, VL], BF, isOutput=True)

    with tile.TileContext(nc) as tc:
        with (
            tc.tile_pool(name="persist", bufs=1) as pp,
            tc.tile_pool(name="stage", bufs=4) as sp,
        ):
            # ---- persistent SBUF tiles ----
            u_sb = pp.tile([128, KC, D], BF, tag="u")
            cwt_sb = pp.tile([128, KC, D], BF, tag="cwt")
            w_sb = pp.tile([128, KC, VL], BF, tag="w")
            ident = pp.tile([128, 128], DT, tag="ident")
            identb = pp.tile([128, 128], BF, tag="identb")
            ones_b = pp.tile([1, 128], BF, tag="ones")
            b2r_sb = pp.tile([1, D], BF, tag="b2r")
            b2c_sb = pp.tile([128, KC], DT, tag="b2c")
            madd_sb = pp.tile([1, B * T], BF, tag="madd")
            # x'/h' layouts: [d_lo, k, b, t]
            xs = pp.tile([128, T, KC, B], BF, tag="xs")
            xt = pp.tile([128, T, KC, B], BF, tag="xt")
            hd2 = pp.tile([128, 2, KC, B], BF, tag="hd2")  # enc l2 state (dbl-buf)
            hdc = pp.tile([128, 2, KC, B], BF, tag="hdc")  # dec state (dbl-buf)
            hd1 = pp.tile([128, KC, B], BF, tag="hd1")  # enc layer-1 state
            he = pp.tile([128, KC, B, T], BF, tag="he")  # enc H'
            hd = pp.tile([128, KC, B, T], BF, tag="hd")  # dec h'
            het = pp.tile([128, B, KC, 128], BF, tag="het")  # He_T[ts, b, k, d]
            ctx = pp.tile([128, KC, B, T], BF, tag="ctx")  # ctx'
            houts = pp.tile([128, KC, B, 2, 64], BF, tag="houts")  # [d,k,b,H,t']

            # ---- load constants ----
            # Spread input DMAs over three issuing queues so the transfers
            # run in parallel: Sync carries what the encoder needs first,
            # DVE carries x_tgt, ACT carries W/cwt/madd (needed later).
            nc.sync.dma_start(out=xs[:, 0:16, :, :], in_=xs_d[:, 0:16, :, :])
            for k in range(KC):
                nc.sync.dma_start(out=u_sb[:, k, :], in_=u_d[k * 128:(k + 1) * 128, :])
            nc.sync.dma_start(out=b2c_sb[:, :], in_=b2c_d[:, :])
            nc.sync.dma_start(out=b2r_sb[:, :], in_=b2r_d[:, :])
            nc.sync.dma_start(out=xs[:, 16:, :, :], in_=xs_d[:, 16:, :, :])
            nc.gpsimd.dma_start(out=xt[:, :, :, :], in_=xt_d[:, :, :, :])
            for k in range(KC):
                nc.sync.dma_start(out=w_sb[:, k, :],
                                  in_=w_d[k * 128:(k + 1) * 128, :])
            for k in range(KC):
                nc.sync.dma_start(out=cwt_sb[:, k, :],
                                  in_=cwt_d[k * 128:(k + 1) * 128, :])
            nc.sync.dma_start(out=madd_sb[:, :], in_=madd_d[:, :])
            make_identity(nc, ident[:, :])
            nc.vector.tensor_copy(out=identb[:, :], in_=ident[:, :])
            nc.gpsimd.memset(ones_b[:, :], 1.0)

            # =============== encoder scan ===============
            with (
                tc.tile_pool(name="pe1", bufs=2, space="PSUM") as pe1,
                tc.tile_pool(name="pe2", bufs=2, space="PSUM") as pe2,
            ):
                for t in range(T):
                    if t == 0:
                        # h1_0 = tanh(x_0)  (x includes b1)
                        nc.scalar.activation(out=hd1[:, :, :], in_=xs[:, 0, :, :],
                                             func=AF.Tanh)
                        # h2_0 = tanh(h1_0 + b2)
                        for m in range(KC):
                            nc.scalar.activation(out=hd2[:, 0, m, :],
                                                 in_=hd1[:, m, :], func=AF.Tanh,
                                                 bias=b2c_sb[:, m:m + 1])
                        nc.gpsimd.tensor_copy(out=he[:, :, :, 0],
                                              in_=hd2[:, 0, :, :])
                        continue
                    # ---- layer 1: p1 = x_t (+b1, folded) + U.T-chunks @ h1 ----
                    p1 = pe1.tile([128, KC, B], DT, tag="p1")
                    nc.tensor.matmul(out=p1[:, :, :], lhsT=identb[:, :],
                                     rhs=xs[:, t, :, :], start=True, stop=False,
                                     skip_group_check=True)
                    for m in range(KC):
                        for k in range(KC):
                            nc.tensor.matmul(
                                out=p1[:, m, :],
                                lhsT=u_sb[:, k, m * 128:(m + 1) * 128],
                                rhs=hd1[:, k, :],
                                start=False, stop=(k == KC - 1),
                                skip_group_check=True)
                    nc.scalar.activation(out=hd1[:, :, :], in_=p1[:, :, :],
                                         func=AF.Tanh)
                    # ---- layer 2: p2 = b2 + h1_t + U.T-chunks @ h2 ----
                    p2 = pe2.tile([128, KC, B], DT, tag="p2")
                    if with_b2:
                        for m in range(KC):
                            nc.tensor.matmul(out=p2[:, m, :],
                                             lhsT=b2r_sb[:, m * 128:(m + 1) * 128],
                                             rhs=ones_b[:, 0:B],
                                             start=(m == 0), stop=False,
                                             skip_group_check=True)
                    for m in range(KC):
                        for k in range(KC):
                            nc.tensor.matmul(
                                out=p2[:, m, :],
                                lhsT=u_sb[:, k, m * 128:(m + 1) * 128],
                                rhs=hd2[:, (t - 1) % 2, k, :],
                                start=(not with_b2 and m == 0 and k == 0),
                                stop=False,
                                skip_group_check=True)
                    nc.tensor.matmul(out=p2[:, :, :], lhsT=identb[:, :],
                                     rhs=hd1[:, :, :], start=False, stop=True,
                                     skip_group_check=True)
                    nc.scalar.activation(out=hd2[:, t % 2, :, :],
                                         in_=p2[:, :, :], func=AF.Tanh)
                    nc.gpsimd.tensor_copy(out=he[:, :, :, t],
                                          in_=hd2[:, t % 2, :, :])

            # =============== decoder scan (+ interleaved He transposes) ======
            with (
                tc.tile_pool(name="pd", bufs=2, space="PSUM") as pd,
                tc.tile_pool(name="pt", bufs=2, space="PSUM") as pt,
            ):
                tp_jobs = [(b, m) for b in range(B) for m in range(KC)]  # 64
                for t in range(T):
                    p = pd.tile([128, KC, B], DT, tag="pdec")
                    nc.tensor.matmul(out=p[:, :, :], lhsT=identb[:, :],
                                     rhs=xt[:, t, :, :], start=True, stop=False,
                                     skip_group_check=True)
                    for m in range(KC):
                        for k in range(KC):
                            prev = (hd2[:, (T - 1) % 2, k, :] if t == 0
                                    else hdc[:, (t - 1) % 2, k, :])
                            nc.tensor.matmul(
                                out=p[:, m, :],
                                lhsT=u_sb[:, k, m * 128:(m + 1) * 128],
                                rhs=prev,
                                start=False, stop=(k == KC - 1),
                                skip_group_check=True)
                    nc.scalar.activation(out=hdc[:, t % 2, :, :],
                                         in_=p[:, :, :], func=AF.Tanh)
                    nc.gpsimd.tensor_copy(out=hd[:, :, :, t],
                                          in_=hdc[:, t % 2, :, :])
                    # one He_T transpose every other step fills PE idle time
                    if t % 2 == 1 and tp_jobs:
                        b, m = tp_jobs.pop(0)
                        tps = pt.tile([128, 128], BF, tag="tps")
                        nc.tensor.transpose(tps[:, :], he[:, m, b, :],
                                            identb[:, :])
                        nc.vector.tensor_copy(out=het[:, b, m, :],
                                              in_=tps[:, :])

            # =============== attention (batched over t) ===============
            with (
                tc.tile_pool(name="ps", bufs=2, space="PSUM") as ps,
                tc.tile_pool(name="pa", bufs=3, space="PSUM") as pa,
                tc.tile_pool(name="pc", bufs=3, space="PSUM") as pc,
                tc.tile_pool(name="watt", bufs=4) as watt,
            ):
                for b in range(B):
                    # scores S[tt, ts] = mask + sum_d hd[d, tt] he[d, ts]
                    s_ps = ps.tile([128, 128], DT, tag="sps")
                    nc.tensor.matmul(out=s_ps[:, :], lhsT=ones_b[:, :],
                                     rhs=madd_sb[:, b * T:(b + 1) * T],
                                     start=True, stop=False,
                                     skip_group_check=True)
                    for k in range(KC):
                        nc.tensor.matmul(out=s_ps[:, :], lhsT=hd[:, k, b, :],
                                         rhs=he[:, k, b, :],
                                         start=False, stop=(k == KC - 1),
                                         skip_group_check=True)
                    # softmax over ts (free axis), scale 1/16 inside exp
                    ex = watt.tile([128, 128], BF, tag="ex")
                    sm = watt.tile([128, 1], DT, tag="sm")
                    nc.scalar.activation(out=ex[:, :], in_=s_ps[:, :],
                                         func=AF.Exp, scale=1.0 / 16.0,
                                         accum_out=sm[:, :])
                    rs = watt.tile([128, 1], DT, tag="rs")
                    nc.vector.reciprocal(out=rs[:, :], in_=sm[:, :])
                    alpha = watt.tile([128, 128], BF, tag="alpha")
                    nc.vector.tensor_scalar(out=alpha[:, :], in0=ex[:, :],
                                            scalar1=rs[:, :1], scalar2=None,
                                            op0=ALU.mult)
                    # alpha [tt, ts] -> alphaT [ts, tt]
                    a_ps = pa.tile([128, 128], BF, tag="aps")
                    nc.tensor.transpose(a_ps[:, :], alpha[:, :], identb[:, :])
                    a_t = watt.tile([128, 128], BF, tag="at")
                    nc.scalar.copy(out=a_t[:, :], in_=a_ps[:, :])
                    # ctx'[d_m, tt] = He_T[ts, d_m].T @ alphaT[ts, tt]
                    for m in range(KC):
                        c_ps = pc.tile([128, 128], DT, tag="cps")
                        nc.tensor.matmul(out=c_ps[:, :], lhsT=het[:, b, m, :],
                                         rhs=a_t[:, :], start=True, stop=True)
                        nc.vector.tensor_copy(out=ctx[:, m, b, :],
                                              in_=c_ps[:, :])

            # ---- outs = hd + ctx @ ctx_W.T  -> houts[d, k, b, H, t'] ----
            with tc.tile_pool(name="po", bufs=2, space="PSUM") as po:
                for m in range(KC):
                    for g in range(8):
                        o_ps = po.tile([128, 512], DT, tag="ops")
                        for k in range(KC):
                            nc.tensor.matmul(
                                out=o_ps[:, :],
                                lhsT=cwt_sb[:, k, m * 128:(m + 1) * 128],
                                rhs=ctx[:, k, g * 4:(g + 1) * 4, :],
                                start=(k == 0), stop=(k == KC - 1))
                        nc.vector.tensor_add(
                            out=houts[:, m, g * 4:(g + 1) * 4, :, :],
                            in0=o_ps[:, :],
                            in1=hd[:, m, g * 4:(g + 1) * 4, :])

            # =============== vocab projection ===============
            with tc.tile_pool(name="pl", bufs=4, space="PSUM") as pl:
                for b in range(B):
                    stg = sp.tile([128, VL], BF, tag="stg")
                    for vb in range(NVB):
                        l_ps = pl.tile([128, VB], DT, tag="lps")
                        for k in range(KC):
                            nc.tensor.matmul(
                                out=l_ps[:, :],
                                lhsT=houts[:, k, b, :, :],
                                rhs=w_sb[:, k, vb * VB:(vb + 1) * VB],
                                start=(k == 0), stop=(k == KC - 1))
                        if vb % 2 == 0:
                            nc.vector.tensor_copy(
                                out=stg[:, vb * VB:(vb + 1) * VB],
                                in_=l_ps[:, :])
                        else:
                            nc.scalar.copy(
                                out=stg[:, vb * VB:(vb + 1) * VB],
                                in_=l_ps[:, :])
                        if vb == 3:
                            nc.sync.dma_start(
                                out=out_d[b * T:(b + 1) * T, 0:4 * VB],
                                in_=stg[:, 0:4 * VB])
                    nc.sync.dma_start(out=out_d[b * T:(b + 1) * T, 4 * VB:],
                                      in_=stg[:, 4 * VB:])
    nc.compile()
    return nc


def _prep_in_maps(U, b_enc1, b_enc2, b_dec, E_en, E_de, ctx_W, W_out_de,
                  src_en, tgt_de_in):
    f32 = np.float32
    U = np.ascontiguousarray(U, f32).astype(NPBF)
    cwt = np.ascontiguousarray(np.asarray(ctx_W, f32).T).astype(NPBF)
    w_t = np.ascontiguousarray(np.asarray(W_out_de, f32).T)  # [D, V] f32
    E_en = np.asarray(E_en, f32)
    E_de = np.asarray(E_de, f32)
    src = np.asarray(src_en)
    tgt = np.asarray(tgt_de_in)

    def gather_x(E, idx, bias):
        # x'[p, k, b, t] = E[idx[t, b], k*128 + p] + bias[k*128+p]
        x = E[idx] + np.asarray(bias, f32)  # (T, B, D)
        x = x.transpose(2, 0, 1).reshape(KC, 128, T, B)  # (k, p, t, b)
        x = x.transpose(1, 2, 0, 3)  # (p, t, k, b)
        return np.ascontiguousarray(x).astype(NPBF)

    xs = gather_x(E_en, src, b_enc1)
    xt = gather_x(E_de, tgt, b_dec)
    b2row = np.asarray(b_enc2, f32).reshape(1, D).astype(NPBF)
    b2col = np.ascontiguousarray(np.asarray(b_enc2, f32).reshape(KC, 128).T)
    madd = np.where(src == 0, f32(-1e9), f32(0.0)).T.reshape(1, B * T)  # (b,t)
    madd = madd.astype(NPBF)

    in_maps = []
    for i in range(NCORES):
        in_maps.append({
            "u": U, "cwt": cwt,
            "w_slice": np.ascontiguousarray(
                w_t[:, i * VL:(i + 1) * VL]).astype(NPBF),
            "xs": xs, "xt": xt,
            "b2row": b2row, "b2col": b2col, "madd": madd,
        })
    return in_maps


def kernel(U, b_enc1, b_enc2, b_dec, E_en, E_de, ctx_W, W_out_de,
           src_en, tgt_de_in, _trace=False, _raw=False):
    with_b2 = bool(np.any(np.asarray(b_enc2) != 0))
    key = ("nc", with_b2)
    if key not in _CACHE:
        _CACHE[key] = _build(with_b2)
    nc = _CACHE[key]
    in_maps = _prep_in_maps(U, b_enc1, b_enc2, b_dec, E_en, E_de, ctx_W,
                            W_out_de, src_en, tgt_de_in)
    res = run_bass_kernel_spmd(nc, in_maps, list(range(NCORES)), trace=_trace)
    if _raw:
        return res
    logits = np.empty((T, B, V), np.float32)
    for i in range(NCORES):
        blk = res.results[i]["out"].astype(np.float32).reshape(B, T, VL)
        logits[:, :, i * VL:(i + 1) * VL] = blk.transpose(1, 0, 2)
    if _trace:
        return logits, res
    return logits
